# revision 1
# baseline (speedup 1.0000x reference)
"""Causal varlen self-attention (qk-norm + rotary + head gating) on 8 trn2 cores.

Sharding: data-parallel by sequence — 8 packed equal-length sequences, one per
NeuronCore; weights replicated. No collectives.

Per-core dataflow (S=1024 tokens, C=1024 hidden, H=16 heads, D=64):
  phase 1: qkv computed TRANSPOSED ([feat, tok]) so scores need no transposes.
           rotary + rms-norm applied in transposed layout (rms scale via
           gpsimd partition-broadcast); v is PE-transposed into natural
           [k_tok, D] layout with a ones column appended (softmax denominator
           falls out of the PV matmul for free). gate = sigmoid(gw @ x^T + b).
  phase 2: per (head, k-tile): scores_T = k_tile^T-stationary x q-moving,
           exp on ACT, causal mask on the diagonal tile only, PV accumulates
           [65, S] (row 64 = denominators). Normalization + gating applied as
           one broadcast multiply on the accumulated attention output.
  phase 3: out^T = Wo^T-tiles-stationary x ao-moving; host transposes back.

All large matmuls run as float32r (full PE rate for moving-dim >= 256).
Compute-engine APs must start at partition 0/32/64/96; per-head stat rows are
routed through base-0 staging tiles + SBUF-to-SBUF DMA (which is unrestricted).
"""

import sys

sys.path.insert(0, "/opt/trn_rl_repo")

import numpy as np
import bass_rust
import concourse.bass as bass
import concourse.tile as tile
from concourse import mybir
from concourse import bass_utils
from concourse.vector_clock import ScopedClock

import os
BCAST_DMA = os.environ.get("KBCAST", "dma") == "dma"
KPHASE = int(os.environ.get("KPHASE", "3"))

P = 128
S = 1024  # tokens per sequence (= per core)
C = 1024  # hidden
H = 16
D = 64
NCORES = 8
F32 = mybir.dt.float32
F32R = mybir.dt.float32r
AF = mybir.ActivationFunctionType


class TC(tile.TileContext):
    """TileContext that rewrites every instruction to carry at most ONE sem wait.

    This container's walrus rejects instructions with more than one sync wait
    command (matmul LDW structs, CTRL drains, ...). Tile's wait-assignment
    pass attaches one wait per producer proc, so fan-in instructions get
    several. After scheduling, hoist all but the last wait of each
    instruction onto same-engine NOPs inserted immediately before it —
    identical synchronization semantics, one wait per encoded instruction.
    """

    _split_seq = 0
    split_waits = True

    def schedule_and_allocate(self, *args, **kwargs):
        ret = super().schedule_and_allocate(*args, **kwargs)
        if not self.split_waits:
            return ret
        nc = self.nc
        for fn in nc.m.functions:
            for blk in fn.blocks:
                insts = blk.instructions
                out = []
                changed = False
                for ins in insts:
                    si = getattr(ins, "sync_info", None)
                    waits = list(si.on_wait) if si is not None else []
                    if len(waits) > 1:
                        changed = True
                        for w in waits[:-1]:
                            TC._split_seq += 1
                            nop = bass_rust.InstNoOp(
                                name=f"I-splitw-{TC._split_seq}",
                                engine=ins.engine,
                                ins=[],
                                outs=[],
                            )
                            nop.sync_info = bass_rust.SyncInfo(
                                on_wait=[w], on_update=[]
                            )
                            out.append(nop)
                        ins.sync_info = bass_rust.SyncInfo(
                            on_wait=[waits[-1]], on_update=list(si.on_update)
                        )
                    out.append(ins)
                if changed:
                    blk.instructions = out
        return ret


def _r(ap):
    return ap.bitcast(F32R)


def build_program(split_waits=True):
    nc = bass.Bass("TRN2", target_bir_lowering=False, debug=False)
    dt = nc.dram_tensor
    xt_d = dt("xt", [C, S], F32R, kind="ExternalInput").ap()
    wqkv_d = dt("wqkv", [24, P, 8, P], F32R, kind="ExternalInput").ap()
    wo_d = dt("wo", [8, P, 8, P], F32R, kind="ExternalInput").ap()
    gw_d = dt("gw", [P, P], F32R, kind="ExternalInput").ap()
    gb_d = dt("gb", [H, 1], F32, kind="ExternalInput").ap()
    cosf_d = dt("cosf", [P, S], F32, kind="ExternalInput").ap()
    sinp_d = dt("sinp", [P, S], F32, kind="ExternalInput").ap()
    maskt_d = dt("maskt", [P, P], F32, kind="ExternalInput").ap()
    bones_d = dt("bones", [P, 2], F32R, kind="ExternalInput").ap()
    ident_d = dt("ident", [64, 64], F32, kind="ExternalInput").ap()
    outt_d = dt("outt", [C, S], F32, kind="ExternalOutput").ap()
    srt_scr = dt("srt_scr", [32, S], F32).ap()
    sums_scr = dt("sums_scr", [H, S], F32).ap()

    with TC(nc) as tc:
        tc.split_waits = split_waits
        with (
            tc.tile_pool(name="const", bufs=1) as constp,
            tc.tile_pool(name="resid", bufs=1) as resid,
            tc.tile_pool(name="stats", bufs=1) as stats,
        ):
            cosf = constp.tile([P, S], F32, tag="cosf")
            sinp = constp.tile([P, S], F32, tag="sinp")
            maskt = constp.tile([P, P], F32, tag="maskt")
            bones = constp.tile([P, 2], F32R, tag="bones")
            ident = constp.tile([64, 64], F32, tag="ident")
            gw_sb = constp.tile([P, P], F32R, tag="gw")
            gb_sb = constp.tile([H, 1], F32, tag="gb")
            nc.sync.dma_start(cosf[:], cosf_d[:])
            nc.sync.dma_start(sinp[:], sinp_d[:])
            nc.sync.dma_start(maskt[:], maskt_d[:])
            nc.sync.dma_start(bones[:], bones_d[:])
            nc.sync.dma_start(ident[:], ident_d[:])
            nc.sync.dma_start(gw_sb[:], gw_d[:])
            nc.sync.dma_start(gb_sb[:], gb_d[:])

            qr = resid.tile([P, 8, S], F32, tag="qr")
            kr = resid.tile([P, 8, S], F32, tag="kr")
            vaug = resid.tile([P, 8, H * 65], F32, tag="vaug")

            gate_sb = stats.tile([H, S], F32, tag="gate")
            sums = stats.tile([H, S], F32, tag="sums")
            srt = stats.tile([32, S], F32, tag="srt")
            eps2q = stats.tile([2, 1], F32, tag="eps2q")
            eps2k = stats.tile([2, 1], F32, tag="eps2k")
            nc.vector.memset(eps2q[:], 1e-6)
            nc.vector.memset(eps2k[:], 6.4e-5)

            # ones columns of v_aug (col 64 of each head's 65-wide block)
            for kt in range(8):
                ones_ap = vaug[:, kt, :].rearrange("p (h e) -> p h e", h=H)[
                    :, :, 64:65
                ]
                nc.vector.memset(ones_ap, 1.0)

            # ---------------- phase 1: projections ----------------
            with (
                tc.tile_pool(name="xp", bufs=1) as xp,
                tc.tile_pool(name="wqks", bufs=2) as wqks,
                tc.tile_pool(name="work", bufs=2) as work,
                tc.tile_pool(name="bcp", bufs=2) as bcp,
                tc.tile_pool(name="stg1", bufs=3) as stg1p,
                tc.tile_pool(name="stg2", bufs=2) as stg2p,
                tc.tile_pool(name="pq", bufs=2, space="PSUM") as pqp,
                tc.tile_pool(name="pgate", bufs=1, space="PSUM") as pgatep,
                tc.tile_pool(name="pbones", bufs=1, space="PSUM") as pbonesp,
                tc.tile_pool(name="pvt", bufs=2, space="PSUM") as pvtp,
            ):
                xT = xp.tile([P, 8, S], F32R, tag="xT")
                for c in range(8):
                    nc.sync.dma_start(xT[:, c, :], xt_d[c * P : (c + 1) * P, :])

                # gate logits, one 512-chunk at a time
                for ch in range(2):
                    sl = slice(ch * 512, (ch + 1) * 512)
                    pgate = pgatep.tile([H, 512], F32, tag="pgate")
                    for c in range(8):
                        nc.tensor.matmul(
                            pgate[:],
                            _r(gw_sb[:, c * H : (c + 1) * H]),
                            _r(xT[:, c, sl]),
                            start=(c == 0),
                            stop=(c == 7),
                        )
                    nc.scalar.activation(
                        gate_sb[:, sl], pgate[:], AF.Sigmoid, bias=gb_sb[:, 0:1]
                    )

                for f in range(24):
                    wt = wqks.tile([P, 8, P], F32R, tag="wt")
                    nc.sync.dma_start(wt[:], wqkv_d[f])
                    pq = pqp.tile([P, S], F32, tag="pq")
                    for c in range(8):
                        for ch in range(2):
                            sl = slice(ch * 512, (ch + 1) * 512)
                            nc.tensor.matmul(
                                pq[:, sl],
                                _r(wt[:, c, :]),
                                _r(xT[:, c, sl]),
                                start=(c == 0),
                                stop=(c == 7),
                            )
                    if f < 16:
                        dst = qr if f < 8 else kr
                        t = f % 8
                        tmp = work.tile([P, S], F32, tag="w1")
                        # rotary (half-split, transposed layout)
                        nc.vector.tensor_mul(_r(dst[:, t, :]), pq[:], cosf[:])
                        for hl in range(2):
                            b0 = hl * 64
                            nc.vector.tensor_mul(
                                tmp[b0 : b0 + 32, :],
                                pq[b0 + 32 : b0 + 64, :],
                                sinp[b0 : b0 + 32, :],
                            )
                            nc.vector.tensor_mul(
                                tmp[b0 + 32 : b0 + 64, :],
                                pq[b0 : b0 + 32, :],
                                sinp[b0 + 32 : b0 + 64, :],
                            )
                        nc.vector.tensor_add(_r(dst[:, t, :]), dst[:, t, :], tmp[:])
                        # sum of squares over D per head -> sqrt -> srt rows
                        sq = work.tile([P, S], F32, tag="w1")
                        nc.vector.tensor_mul(_r(sq[:]), dst[:, t, :], dst[:, t, :])
                        ro = (0 if f < 8 else 16) + 2 * t
                        for ch in range(2):
                            sl = slice(ch * 512, (ch + 1) * 512)
                            pb = pbonesp.tile([2, 512], F32, tag="pb")
                            nc.tensor.matmul(pb[:], _r(bones[:]), _r(sq[:, sl]))
                            s2 = stg2p.tile([2, 512], F32, tag="s2")
                            if f < 8:
                                nc.scalar.activation(
                                    s2[:], pb[:], AF.Sqrt, bias=eps2q[:, 0:1],
                                    scale=1.0 / 64,
                                )
                            else:
                                nc.scalar.activation(
                                    s2[:], pb[:], AF.Sqrt, bias=eps2k[:, 0:1],
                                    scale=1.0,
                                )
                            nc.sync.dma_start(srt[ro : ro + 2, sl], s2[:])
                    else:
                        # v: evacuate both head halves to base-0 tiles, then
                        # PE-transpose each k-tile into natural layout
                        tv = f - 16
                        va = work.tile([64, S], F32, tag="va")
                        vb = work.tile([64, S], F32, tag="vb")
                        nc.vector.tensor_copy(va[:], pq[0:64, :])
                        nc.vector.tensor_copy(vb[:], pq[64:128, :])
                        for kt in range(8):
                            for hl, vh in ((0, va), (1, vb)):
                                pvt = pvtp.tile([P, 64], F32, tag="pvt")
                                nc.tensor.transpose(
                                    pvt[:],
                                    vh[:, kt * P : (kt + 1) * P],
                                    ident[0:64, :],
                                )
                                h = 2 * tv + hl
                                nc.vector.tensor_copy(
                                    _r(vaug[:, kt, h * 65 : h * 65 + 64]), pvt[:]
                                )

                # reciprocal of all 32 sqrt rows, then apply per feat-tile
                nc.vector.reciprocal(srt[:], srt[:])
                nc.sync.dma_start(srt_scr[:, :], srt[:])
                for side, dst in ((0, qr), (16, kr)):
                    for t in range(8):
                        bc = bcp.tile([P, S], F32, tag="bc")
                        for hl in range(2):
                            ro = side + 2 * t + hl
                            if BCAST_DMA:
                                nc.sync.dma_start(
                                    bc[hl * 64 : (hl + 1) * 64, :],
                                    srt_scr[ro : ro + 1, :].broadcast_to([64, S]),
                                )
                            else:
                                nc.vector.memset(bc[hl * 64 : (hl + 1) * 64, :], 1.0)
                        nc.vector.tensor_mul(_r(dst[:, t, :]), dst[:, t, :], bc[:])

            # ---------------- phases 2+3 ----------------
            if KPHASE < 2:
                nc.sync.dma_start(outt_d[:, :], qr[:])
                return nc
            with tc.tile_pool(name="aop", bufs=1) as aop:
                aos = aop.tile([P, 8, S], F32, tag="aos")
                # ---------------- phase 2: attention ----------------
                with (
                    tc.tile_pool(name="expp", bufs=3) as expp,
                    tc.tile_pool(name="bc2", bufs=2) as bc2p,
                    tc.tile_pool(name="st2", bufs=3) as st2p,
                    tc.tile_pool(name="ps", bufs=2, space="PSUM") as psp,
                    tc.tile_pool(name="po", bufs=2, space="PSUM") as pop,
                ):
                    for h in range(H):
                        ft, r0 = h // 2, (h % 2) * 64
                        po = pop.tile([65, S], F32, tag="po")
                        for kt in range(8):
                            q0 = kt * P
                            nsp = S - q0
                            et = expp.tile([P, S], F32, tag="et")
                            ofs = 0
                            while ofs < nsp:
                                n = min(512, nsp - ofs)
                                ps = psp.tile([P, 512], F32, tag="ps")
                                nc.tensor.matmul(
                                    ps[:, 0:n],
                                    _r(kr[r0 : r0 + 64, ft, q0 : q0 + P]),
                                    _r(qr[r0 : r0 + 64, ft, q0 + ofs : q0 + ofs + n]),
                                )
                                nc.scalar.activation(
                                    _r(et[:, ofs : ofs + n]), ps[:, 0:n], AF.Exp
                                )
                                ofs += n
                            # causal mask on the diagonal tile
                            nc.vector.tensor_mul(_r(et[:, 0:P]), et[:, 0:P], maskt[:])
                            ofs = 0
                            while ofs < nsp:
                                a = q0 + ofs
                                n = min(512 - (a % 512), nsp - ofs)
                                nc.tensor.matmul(
                                    po[:, a : a + n],
                                    _r(vaug[:, kt, h * 65 : (h + 1) * 65]),
                                    _r(et[:, ofs : ofs + n]),
                                    start=(kt == 0),
                                    stop=(kt == 4 * (a // 512) + 3),
                                )
                                ofs += n
                        # denominator row -> sums[h] via base-0 staging + DMA
                        s1 = st2p.tile([1, S], F32, tag="sd")
                        nc.scalar.activation(s1[:], po[64:65, :], AF.Copy)
                        nc.sync.dma_start(sums[h : h + 1, :], s1[:])
                        nc.vector.tensor_copy(_r(aos[r0 : r0 + 64, ft, :]), po[0:64, :])

                    # scale = gate / denominator, applied per channel-tile
                    nc.vector.reciprocal(sums[:], sums[:])
                    nc.vector.tensor_mul(sums[:], sums[:], gate_sb[:])
                    nc.sync.dma_start(sums_scr[:, :], sums[:])
                    for ct in range(8):
                        bc = bc2p.tile([P, S], F32, tag="bc2")
                        for hl in range(2):
                            ro = 2 * ct + hl
                            if BCAST_DMA:
                                nc.sync.dma_start(
                                    bc[hl * 64 : (hl + 1) * 64, :],
                                    sums_scr[ro : ro + 1, :].broadcast_to([64, S]),
                                )
                            else:
                                nc.vector.memset(bc[hl * 64 : (hl + 1) * 64, :], 1.0)
                        nc.vector.tensor_mul(_r(aos[:, ct, :]), aos[:, ct, :], bc[:])

                if KPHASE < 3:
                    nc.sync.dma_start(outt_d[:, :], aos[:])
                    return nc
                # ---------------- phase 3: output projection ----------------
                with (
                    tc.tile_pool(name="wop", bufs=2) as wop,
                    tc.tile_pool(name="osb", bufs=2) as osbp,
                    tc.tile_pool(name="pw", bufs=2, space="PSUM") as pwp,
                ):
                    for o in range(8):
                        wt = wop.tile([P, 8, P], F32R, tag="wo")
                        nc.sync.dma_start(wt[:], wo_d[o])
                        pw = pwp.tile([P, S], F32, tag="pw")
                        for c in range(8):
                            for ch in range(2):
                                sl = slice(ch * 512, (ch + 1) * 512)
                                nc.tensor.matmul(
                                    pw[:, sl],
                                    _r(wt[:, c, :]),
                                    _r(aos[:, c, sl]),
                                    start=(c == 0),
                                    stop=(c == 7),
                                )
                        ot = osbp.tile([P, S], F32, tag="ot")
                        nc.scalar.activation(ot[:], pw[:], AF.Copy)
                        nc.sync.dma_start(outt_d[o * P : (o + 1) * P, :], ot[:])
    return nc


def prepare_inputs(x, Wqkv, Wo, gate_w, gate_b, cos_cache, sin_cache, position_ids):
    """Host-side sharding + layout prep. Returns per-core input maps."""
    x = np.asarray(x, dtype=np.float32)
    WqkvT = np.asarray(Wqkv, dtype=np.float32).T  # [C, 3C]
    wqkv_r = np.ascontiguousarray(
        WqkvT.reshape(8, P, 24, P).transpose(2, 1, 0, 3)
    )  # [f, p, c, d]
    WoT = np.asarray(Wo, dtype=np.float32).T  # [C, C]
    wo_r = np.ascontiguousarray(WoT.reshape(8, P, 8, P).transpose(2, 1, 0, 3))
    gwT = np.asarray(gate_w, dtype=np.float32).T  # [C, H]
    gw_r = np.ascontiguousarray(
        gwT.reshape(8, P, H).transpose(1, 0, 2).reshape(P, P)
    )
    gb_r = np.asarray(gate_b, dtype=np.float32).reshape(H, 1)
    maskt = np.triu(np.ones((P, P), dtype=np.float32))
    bones = np.zeros((P, 2), dtype=np.float32)
    bones[0:64, 0] = 1.0
    bones[64:128, 1] = 1.0
    ident = np.eye(64, dtype=np.float32)
    cos_cache = np.asarray(cos_cache, dtype=np.float32)
    sin_cache = np.asarray(sin_cache, dtype=np.float32)
    position_ids = np.asarray(position_ids)

    in_maps = []
    for b in range(NCORES):
        xs = x[b * S : (b + 1) * S, :]
        pos = position_ids[b * S : (b + 1) * S]
        ct = cos_cache[pos].T  # [32, S]
        st = sin_cache[pos].T
        cosf = np.ascontiguousarray(np.tile(ct, (4, 1)))
        sinp = np.ascontiguousarray(
            np.tile(np.concatenate([st, -st], axis=0), (2, 1))
        )
        in_maps.append(
            {
                "xt": np.ascontiguousarray(xs.T),
                "wqkv": wqkv_r,
                "wo": wo_r,
                "gw": gw_r,
                "gb": gb_r,
                "cosf": cosf,
                "sinp": sinp,
                "maskt": maskt,
                "bones": bones,
                "ident": ident,
            }
        )
    return in_maps


_CACHED_NC = None


def kernel(
    x,
    Wqkv,
    Wo,
    gate_w,
    gate_b,
    cos_cache,
    sin_cache,
    cu_seqlens,
    position_ids,
    max_seqlen,
):
    global _CACHED_NC
    in_maps = prepare_inputs(
        x, Wqkv, Wo, gate_w, gate_b, cos_cache, sin_cache, position_ids
    )
    if _CACHED_NC is None:
        _CACHED_NC = build_program()
    res = bass_utils.run_bass_kernel_spmd(
        _CACHED_NC, in_maps, core_ids=list(range(NCORES))
    )
    out = np.empty((NCORES * S, C), dtype=np.float32)
    for b in range(NCORES):
        out[b * S : (b + 1) * S, :] = res.results[b]["outt"].T
    return out



# revision 6
# speedup vs baseline: 1.2577x; 1.2577x over previous
"""Causal varlen self-attention (qk-norm + rotary + head gating) on 8 trn2 cores.

Sharding: data-parallel by sequence - 8 packed equal-length sequences, one per
NeuronCore; weights replicated. No collectives.

Per-core dataflow (S=1024 tokens, C=1024 hidden, H=16 heads, D=64):
  phase 1a: q/k computed TRANSPOSED ([feat, tok]) so scores need no transposes.
            rotary + rms-norm applied in transposed layout; sum-of-squares via
            a 65-col padded "bones" stationary (so the PE never leaves 128x128
            tiling mode); rms scales broadcast via SBUF-DMA and applied with
            muls split across Vector and GpSimd.
  phase 1b: v computed in NATURAL [tok, D] layout directly (x-tile stationary,
            Wv moving) - no PE transposes. Ones column appended per head so
            the softmax denominator falls out of the PV matmul. gate =
            sigmoid(gwp @ x^T + b) with gwp padded to 65 cols.
  phase 2:  per head: burst all-kt scores (64x128 row-tile mode), exp on ACT
            into bf16 et tiles ([128,1024] PSUM reads = fat ACT instrs),
            causal mask on the diagonal tile only, then burst all-kt PV
            (bf16, 65-col stationary; row 64 = denominators). Normalization +
            gating applied as one broadcast multiply on the accumulated
            attention output.
  phase 3:  out^T = Wo^T-tiles-stationary x ao-moving; host transposes back.

Matmuls run as float32r except the PV stage (bf16 weights+probs, fp32 PSUM).
Compute-engine APs must start at partition 0/32/64/96; per-head stat rows are
routed through base-0 staging tiles + SBUF-to-SBUF DMA (which is unrestricted).
"""

import sys

sys.path.insert(0, "/opt/trn_rl_repo")

import numpy as np
import bass_rust
import concourse.bass as bass
import concourse.tile as tile
from concourse import mybir
from concourse import bass_utils

import os
KPHASE = int(os.environ.get("KPHASE", "3"))

P = 128
S = 1024  # tokens per sequence (= per core)
C = 1024  # hidden
H = 16
D = 64
NCORES = 8
F32 = mybir.dt.float32
F32R = mybir.dt.float32r
BF16 = mybir.dt.bfloat16
AF = mybir.ActivationFunctionType


class TC(tile.TileContext):
    """TileContext that rewrites every instruction to carry at most ONE sem wait.

    This container's walrus rejects instructions with more than one sync wait
    command (matmul LDW structs, CTRL drains, ...). Tile's wait-assignment
    pass attaches one wait per producer proc, so fan-in instructions get
    several. After scheduling, hoist all but the last wait of each
    instruction onto same-engine NOPs inserted immediately before it -
    identical synchronization semantics, one wait per encoded instruction.
    """

    _split_seq = 0
    split_waits = True

    def schedule_and_allocate(self, *args, **kwargs):
        ret = super().schedule_and_allocate(*args, **kwargs)
        if not self.split_waits:
            return ret
        nc = self.nc
        for fn in nc.m.functions:
            for blk in fn.blocks:
                insts = blk.instructions
                out = []
                changed = False
                for ins in insts:
                    si = getattr(ins, "sync_info", None)
                    waits = list(si.on_wait) if si is not None else []
                    if len(waits) > 1:
                        changed = True
                        for w in waits[:-1]:
                            TC._split_seq += 1
                            nop = bass_rust.InstNoOp(
                                name=f"I-splitw-{TC._split_seq}",
                                engine=ins.engine,
                                ins=[],
                                outs=[],
                            )
                            nop.sync_info = bass_rust.SyncInfo(
                                on_wait=[w], on_update=[]
                            )
                            out.append(nop)
                        ins.sync_info = bass_rust.SyncInfo(
                            on_wait=[waits[-1]], on_update=list(si.on_update)
                        )
                    out.append(ins)
                if changed:
                    blk.instructions = out
        return ret


def _r(ap):
    return ap.bitcast(F32R)


def build_program(split_waits=True):
    nc = bass.Bass("TRN2", target_bir_lowering=False, debug=False)
    dt = nc.dram_tensor
    xt_d = dt("xt", [C, S], F32R, kind="ExternalInput").ap()
    wqk_d = dt("wqk", [16, P, 8, P], F32R, kind="ExternalInput").ap()
    wv_d = dt("wv", [8, P, C], F32R, kind="ExternalInput").ap()
    wo_d = dt("wo", [8, P, 8, P], F32R, kind="ExternalInput").ap()
    gwp_d = dt("gwp", [P, 8, 65], F32R, kind="ExternalInput").ap()
    gb_d = dt("gb", [H, 1], F32, kind="ExternalInput").ap()
    cosf_d = dt("cosf", [P, S], F32, kind="ExternalInput").ap()
    sinp_d = dt("sinp", [P, S], F32, kind="ExternalInput").ap()
    maskt_d = dt("maskt", [P, P], BF16, kind="ExternalInput").ap()
    bonesp_d = dt("bonesp", [P, 65], F32R, kind="ExternalInput").ap()
    outt_d = dt("outt", [C, S], F32, kind="ExternalOutput").ap()
    srt_scr = dt("srt_scr", [32, S], F32).ap()
    sums_scr = dt("sums_scr", [H, S], F32).ap()

    with TC(nc) as tc:
        tc.split_waits = split_waits
        with (
            tc.tile_pool(name="const", bufs=1) as constp,
            tc.tile_pool(name="resid", bufs=1) as resid,
            tc.tile_pool(name="stats", bufs=1) as stats,
        ):
            cosf = constp.tile([P, S], F32, tag="cosf")
            sinp = constp.tile([P, S], F32, tag="sinp")
            maskt = constp.tile([P, P], BF16, tag="maskt")
            bonesp = constp.tile([P, 65], F32R, tag="bonesp")
            gwp_sb = constp.tile([P, 8, 65], F32R, tag="gwp")
            gb_sb = constp.tile([H, 1], F32, tag="gb")
            nc.sync.dma_start(cosf[:], cosf_d[:])
            nc.sync.dma_start(sinp[:], sinp_d[:])
            nc.sync.dma_start(maskt[:], maskt_d[:])
            nc.sync.dma_start(bonesp[:], bonesp_d[:])
            nc.sync.dma_start(gwp_sb[:], gwp_d[:])
            nc.sync.dma_start(gb_sb[:], gb_d[:])

            qr = resid.tile([P, 8, S], F32, tag="qr")
            kr = resid.tile([P, 8, S], F32, tag="kr")
            # per k-tile: 16 heads x (64 v-cols + ones col)
            vaug = resid.tile([P, 8, H * 65], BF16, tag="vaug")

            gate_sb = stats.tile([H, S], F32, tag="gate")
            sums = stats.tile([H, S], F32, tag="sums")
            srt = stats.tile([32, S], F32, tag="srt")
            eps2q = stats.tile([2, 1], F32, tag="eps2q")
            eps2k = stats.tile([2, 1], F32, tag="eps2k")
            nc.vector.memset(eps2q[:], 1e-6)
            nc.vector.memset(eps2k[:], 6.4e-5)

            # ones columns of v_aug (col 64 of each head's 65-wide block)
            for kt in range(8):
                ones_ap = vaug[:, kt, :].rearrange("p (h e) -> p h e", h=H)[
                    :, :, 64:65
                ]
                nc.vector.memset(ones_ap, 1.0)

            xp = tc.alloc_tile_pool(name="xp", bufs=1)
            xT = xp.tile([P, 8, S], F32R, tag="xT")
            for c in range(8):
                nc.sync.dma_start(xT[:, c, :], xt_d[c * P : (c + 1) * P, :])

            # ---------------- phase 1a: q/k projections ----------------
            with (
                tc.tile_pool(name="wqks", bufs=2) as wqks,
                tc.tile_pool(name="work", bufs=2) as work,
                tc.tile_pool(name="bcp", bufs=2) as bcp,
                tc.tile_pool(name="stg2", bufs=2) as stg2p,
                tc.tile_pool(name="pq", bufs=2, space="PSUM") as pqp,
                tc.tile_pool(name="pbones", bufs=2, space="PSUM") as pbonesp,
            ):
                for f in range(16):
                    wt = wqks.tile([P, 8, P], F32R, tag="wt")
                    nc.sync.dma_start(wt[:], wqk_d[f])
                    pq = pqp.tile([P, S], F32, tag="pq")
                    for c in range(8):
                        for ch in range(2):
                            sl = slice(ch * 512, (ch + 1) * 512)
                            nc.tensor.matmul(
                                pq[:, sl],
                                _r(wt[:, c, :]),
                                _r(xT[:, c, sl]),
                                start=(c == 0),
                                stop=(c == 7),
                            )
                    dst = qr if f < 8 else kr
                    t = f % 8
                    tmp = work.tile([P, S], F32, tag="w1")
                    # rotary (half-split, transposed layout)
                    nc.vector.tensor_mul(_r(dst[:, t, :]), pq[:], cosf[:])
                    for hl in range(2):
                        b0 = hl * 64
                        nc.vector.tensor_mul(
                            tmp[b0 : b0 + 32, :],
                            pq[b0 + 32 : b0 + 64, :],
                            sinp[b0 : b0 + 32, :],
                        )
                        nc.vector.tensor_mul(
                            tmp[b0 + 32 : b0 + 64, :],
                            pq[b0 : b0 + 32, :],
                            sinp[b0 + 32 : b0 + 64, :],
                        )
                    nc.vector.tensor_add(_r(dst[:, t, :]), dst[:, t, :], tmp[:])
                    # sum of squares over D per head (gpsimd) -> sqrt -> srt
                    sq = work.tile([P, S], F32, tag="w1")
                    nc.vector.tensor_mul(_r(sq[:]), dst[:, t, :], dst[:, t, :])
                    ro = (0 if f < 8 else 16) + 2 * t
                    for ch in range(2):
                        sl = slice(ch * 512, (ch + 1) * 512)
                        pb = pbonesp.tile([65, 512], F32, tag="pb")
                        nc.tensor.matmul(pb[:], _r(bonesp[:]), _r(sq[:, sl]))
                        s2 = stg2p.tile([2, 512], F32, tag="s2")
                        if f < 8:
                            nc.scalar.activation(
                                s2[:], pb[0:2, :], AF.Sqrt, bias=eps2q[:, 0:1],
                                scale=1.0 / 64,
                            )
                        else:
                            nc.scalar.activation(
                                s2[:], pb[0:2, :], AF.Sqrt, bias=eps2k[:, 0:1],
                                scale=1.0,
                            )
                        nc.sync.dma_start(srt[ro : ro + 2, sl], s2[:])

                # reciprocal of all 32 sqrt rows, then apply per feat-tile
                nc.vector.reciprocal(srt[:], srt[:])
                nc.sync.dma_start(srt_scr[:, :], srt[:])
                for side, dst in ((0, qr), (16, kr)):
                    for t in range(8):
                        bc = bcp.tile([P, S], F32, tag="bc")
                        for hl in range(2):
                            ro = side + 2 * t + hl
                            nc.sync.dma_start(
                                bc[hl * 64 : (hl + 1) * 64, :],
                                srt_scr[ro : ro + 1, :].broadcast_to([64, S]),
                            )
                        nc.vector.tensor_mul(
                            _r(dst[:, t, :]), dst[:, t, :], bc[:]
                        )

            # ---------------- phase 1b: v (natural layout) + gate ----------
            with (
                tc.tile_pool(name="wvp", bufs=1) as wvp,
                tc.tile_pool(name="pvn", bufs=4, space="PSUM") as pvnp,
                tc.tile_pool(name="pgate", bufs=2, space="PSUM") as pgatep,
            ):
                wv_sb = wvp.tile([P, 8, C], F32R, tag="wv")
                for c in range(8):
                    nc.sync.dma_start(wv_sb[:, c, :], wv_d[c])

                # gate logits, one 512-chunk at a time (padded 65-col weights)
                for ch in range(2):
                    sl = slice(ch * 512, (ch + 1) * 512)
                    pgate = pgatep.tile([65, 512], F32, tag="pgate")
                    for c in range(8):
                        nc.tensor.matmul(
                            pgate[:],
                            gwp_sb[:, c, :],
                            _r(xT[:, c, sl]),
                            start=(c == 0),
                            stop=(c == 7),
                        )
                    nc.scalar.activation(
                        gate_sb[:, sl], pgate[0:H, :], AF.Sigmoid,
                        bias=gb_sb[:, 0:1],
                    )

                # v[tok, vfeat]: stationary = x tile, moving = Wv chunk
                for tt in range(8):
                    tsl = slice(tt * P, (tt + 1) * P)
                    pva = pvnp.tile([P, 512], F32, tag="pv")
                    pvb = pvnp.tile([P, 512], F32, tag="pv")
                    for c in range(8):
                        nc.tensor.matmul(
                            pva[:],
                            _r(xT[:, c, tsl]),
                            _r(wv_sb[:, c, 0:512]),
                            start=(c == 0),
                            stop=(c == 7),
                        )
                        nc.tensor.matmul(
                            pvb[:],
                            _r(xT[:, c, tsl]),
                            _r(wv_sb[:, c, 512:1024]),
                            start=(c == 0),
                            stop=(c == 7),
                        )
                    vre = vaug[:, tt, :].rearrange("p (h e) -> p h e", h=H)
                    nc.vector.tensor_copy(
                        vre[:, 0:8, 0:64], pva[:].rearrange("p (h e) -> p h e", h=8)
                    )
                    nc.vector.tensor_copy(
                        vre[:, 8:16, 0:64], pvb[:].rearrange("p (h e) -> p h e", h=8)
                    )
            xp.release()

            # ---------------- phases 2+3 ----------------
            if KPHASE < 2:
                nc.sync.dma_start(outt_d[:, :], qr[:])
                return nc
            with tc.tile_pool(name="aop", bufs=1) as aop:
                aos = aop.tile([P, 8, S], F32, tag="aos")
                # ---------------- phase 2: attention ----------------
                with (
                    tc.tile_pool(name="expp", bufs=10) as expp,
                    tc.tile_pool(name="bc2", bufs=2) as bc2p,
                    tc.tile_pool(name="st2", bufs=3) as st2p,
                    tc.tile_pool(name="ps", bufs=2, space="PSUM") as psp,
                    tc.tile_pool(name="po", bufs=2, space="PSUM") as pop,
                ):
                    for h in range(H):
                        ft, r0 = h // 2, (h % 2) * 64
                        po = pop.tile([65, S], F32, tag="po")
                        ets = []
                        # burst: all scores + exp for this head
                        for kt in range(8):
                            q0 = kt * P
                            nsp = S - q0
                            ps = psp.tile([P, S], F32, tag="ps")
                            ofs = 0
                            while ofs < nsp:
                                n = min(512, nsp - ofs)
                                nc.tensor.matmul(
                                    ps[:, ofs : ofs + n],
                                    _r(kr[r0 : r0 + 64, ft, q0 : q0 + P]),
                                    _r(qr[r0 : r0 + 64, ft, q0 + ofs : q0 + ofs + n]),
                                )
                                ofs += n
                            et = expp.tile([P, S], BF16, tag="et")
                            nc.scalar.activation(
                                et[:, 0:nsp], ps[:, 0:nsp], AF.Exp
                            )
                            # causal mask on the diagonal tile
                            nc.vector.tensor_mul(
                                et[:, 0:P], et[:, 0:P], maskt[:]
                            )
                            ets.append((et, nsp, q0))
                        # burst: all PV for this head
                        for kt in range(8):
                            et, nsp, q0 = ets[kt]
                            ofs = 0
                            while ofs < nsp:
                                a = q0 + ofs
                                n = min(512 - (a % 512), nsp - ofs)
                                nc.tensor.matmul(
                                    po[:, a : a + n],
                                    vaug[:, kt, h * 65 : (h + 1) * 65],
                                    et[:, ofs : ofs + n],
                                    start=(kt == 0),
                                    stop=(kt == 4 * (a // 512) + 3),
                                )
                                ofs += n
                        # denominator row -> sums[h] via base-0 staging + DMA
                        s1 = st2p.tile([1, S], F32, tag="sd")
                        nc.vector.tensor_copy(s1[:], po[64:65, :])
                        nc.sync.dma_start(sums[h : h + 1, :], s1[:])
                        nc.vector.tensor_copy(
                            _r(aos[r0 : r0 + 64, ft, :]), po[0:64, :]
                        )

                    # scale = gate / denominator, applied per channel-tile
                    nc.vector.reciprocal(sums[:], sums[:])
                    nc.vector.tensor_mul(sums[:], sums[:], gate_sb[:])
                    nc.sync.dma_start(sums_scr[:, :], sums[:])
                    for ct in range(8):
                        bc = bc2p.tile([P, S], F32, tag="bc2")
                        for hl in range(2):
                            ro = 2 * ct + hl
                            nc.sync.dma_start(
                                bc[hl * 64 : (hl + 1) * 64, :],
                                sums_scr[ro : ro + 1, :].broadcast_to([64, S]),
                            )
                        nc.vector.tensor_mul(
                            _r(aos[:, ct, :]), aos[:, ct, :], bc[:]
                        )

                if KPHASE < 3:
                    nc.sync.dma_start(outt_d[:, :], aos[:])
                    return nc
                # ---------------- phase 3: output projection ----------------
                with (
                    tc.tile_pool(name="wop", bufs=2) as wop,
                    tc.tile_pool(name="osb", bufs=2) as osbp,
                    tc.tile_pool(name="pw", bufs=2, space="PSUM") as pwp,
                ):
                    for o in range(8):
                        wt = wop.tile([P, 8, P], F32R, tag="wo")
                        nc.sync.dma_start(wt[:], wo_d[o])
                        pw = pwp.tile([P, S], F32, tag="pw")
                        for c in range(8):
                            for ch in range(2):
                                sl = slice(ch * 512, (ch + 1) * 512)
                                nc.tensor.matmul(
                                    pw[:, sl],
                                    _r(wt[:, c, :]),
                                    _r(aos[:, c, sl]),
                                    start=(c == 0),
                                    stop=(c == 7),
                                )
                        ot = osbp.tile([P, S], F32, tag="ot")
                        nc.scalar.activation(ot[:], pw[:], AF.Copy)
                        nc.sync.dma_start(outt_d[o * P : (o + 1) * P, :], ot[:])
    return nc


def prepare_inputs(x, Wqkv, Wo, gate_w, gate_b, cos_cache, sin_cache, position_ids):
    """Host-side sharding + layout prep. Returns per-core input maps."""
    x = np.asarray(x, dtype=np.float32)
    WqkvT = np.asarray(Wqkv, dtype=np.float32).T  # [C, 3C]
    wqk_r = np.ascontiguousarray(
        WqkvT[:, 0:2048].reshape(8, P, 16, P).transpose(2, 1, 0, 3)
    )  # [f, p, c, d] for q,k only
    wv_r = np.ascontiguousarray(WqkvT[:, 2048:3072].reshape(8, P, C))
    WoT = np.asarray(Wo, dtype=np.float32).T  # [C, C]
    wo_r = np.ascontiguousarray(WoT.reshape(8, P, 8, P).transpose(2, 1, 0, 3))
    gwT = np.asarray(gate_w, dtype=np.float32).T  # [C, H]
    gwp_r = np.zeros((P, 8, 65), dtype=np.float32)
    gwp_r[:, :, 0:H] = gwT.reshape(8, P, H).transpose(1, 0, 2)
    gb_r = np.asarray(gate_b, dtype=np.float32).reshape(H, 1)

    import ml_dtypes

    maskt = np.triu(np.ones((P, P), dtype=np.float32)).astype(ml_dtypes.bfloat16)
    bonesp = np.zeros((P, 65), dtype=np.float32)
    bonesp[0:64, 0] = 1.0
    bonesp[64:128, 1] = 1.0
    cos_cache = np.asarray(cos_cache, dtype=np.float32)
    sin_cache = np.asarray(sin_cache, dtype=np.float32)
    position_ids = np.asarray(position_ids)

    in_maps = []
    for b in range(NCORES):
        xs = x[b * S : (b + 1) * S, :]
        pos = position_ids[b * S : (b + 1) * S]
        ct = cos_cache[pos].T  # [32, S]
        st = sin_cache[pos].T
        cosf = np.ascontiguousarray(np.tile(ct, (4, 1)))
        sinp = np.ascontiguousarray(
            np.tile(np.concatenate([st, -st], axis=0), (2, 1))
        )
        in_maps.append(
            {
                "xt": np.ascontiguousarray(xs.T),
                "wqk": wqk_r,
                "wv": wv_r,
                "wo": wo_r,
                "gwp": gwp_r,
                "gb": gb_r,
                "cosf": cosf,
                "sinp": sinp,
                "maskt": maskt,
                "bonesp": bonesp,
            }
        )
    return in_maps


_CACHED_NC = None


def kernel(
    x,
    Wqkv,
    Wo,
    gate_w,
    gate_b,
    cos_cache,
    sin_cache,
    cu_seqlens,
    position_ids,
    max_seqlen,
):
    global _CACHED_NC
    in_maps = prepare_inputs(
        x, Wqkv, Wo, gate_w, gate_b, cos_cache, sin_cache, position_ids
    )
    if _CACHED_NC is None:
        _CACHED_NC = build_program()
    res = bass_utils.run_bass_kernel_spmd(
        _CACHED_NC, in_maps, core_ids=list(range(NCORES))
    )
    out = np.empty((NCORES * S, C), dtype=np.float32)
    for b in range(NCORES):
        out[b * S : (b + 1) * S, :] = res.results[b]["outt"].T
    return out


# revision 15
# speedup vs baseline: 1.7499x; 1.3913x over previous
"""Causal varlen self-attention (qk-norm + rotary + head gating) on 8 trn2 cores.

Sharding: data-parallel by sequence - 8 packed equal-length sequences, one per
NeuronCore; weights replicated. No collectives.

Per-core dataflow (S=1024 tokens, C=1024 hidden, H=16 heads, D=64):
  phase 1a: q/k computed TRANSPOSED ([feat, tok]) so scores need no transposes.
            PSUM evacuated to bf16 by ACT; rotary + rms-norm run in bf16 on
            DVE (2x mode); sum-of-squares on ACT (Square) feeding a 65-col
            padded "bones" stationary so the PE never leaves 128x128 mode.
  phase 1b: v computed in NATURAL [tok, D] layout directly (x-tile stationary,
            Wv moving) - no PE transposes. Ones column appended per head so
            the softmax denominator falls out of the PV matmul. gate =
            sigmoid(gwp @ x^T + b) with gwp padded to 65 cols. rms scales are
            applied while v runs on the PE.
  phase 2:  per head PAIR: row-tiled scores (heads 2t/2t+1 run concurrently
            on PE row-groups 0/64), exp on ACT into bf16 et tiles
            ([128,1024] PSUM reads = fat ACT instrs), causal mask on the
            diagonal tile, then PV bursts (bf16, 65-col stationary; row 64 =
            denominators). Normalization + gating applied as one broadcast
            multiply per token-half on the accumulated attention output.
  phase 3:  out^T = Wo^T-tiles-stationary x ao-moving, split by token halves
            so Wo overlaps the gating tail; host transposes back.

Projections/bones run fp32r; q/k/scores/PV/Wo run bf16 (fp32 PSUM accum).
Compute-engine APs must start at partition 0/32/64/96; per-head stat rows are
routed through base-0 staging tiles + SBUF-to-SBUF DMA (which is unrestricted).
"""

import sys

sys.path.insert(0, "/opt/trn_rl_repo")

import numpy as np
import bass_rust
import concourse.bass as bass
import concourse.tile as tile
from concourse import mybir
from concourse import bass_utils

import os
KPHASE = int(os.environ.get("KPHASE", "3"))
KDBG = int(os.environ.get("KDBG", "0"))

P = 128
S = 1024  # tokens per sequence (= per core)
C = 1024  # hidden
H = 16
D = 64
NCORES = 8
F32 = mybir.dt.float32
F32R = mybir.dt.float32r
BF16 = mybir.dt.bfloat16
AF = mybir.ActivationFunctionType


class TC(tile.TileContext):
    """TileContext that rewrites every instruction to carry at most ONE sem wait.

    This container's walrus rejects instructions with more than one sync wait
    command (matmul LDW structs, CTRL drains, ...). Tile's wait-assignment
    pass attaches one wait per producer proc, so fan-in instructions get
    several. After scheduling, hoist all but the last wait of each
    instruction onto same-engine NOPs inserted immediately before it -
    identical synchronization semantics, one wait per encoded instruction.
    """

    _split_seq = 0
    split_waits = True

    def schedule_and_allocate(self, *args, **kwargs):
        ret = super().schedule_and_allocate(*args, **kwargs)
        if not self.split_waits:
            return ret
        nc = self.nc
        for fn in nc.m.functions:
            for blk in fn.blocks:
                insts = blk.instructions
                out = []
                changed = False
                for ins in insts:
                    si = getattr(ins, "sync_info", None)
                    waits = list(si.on_wait) if si is not None else []
                    if len(waits) > 1:
                        changed = True
                        for w in waits[:-1]:
                            TC._split_seq += 1
                            nop = bass_rust.InstNoOp(
                                name=f"I-splitw-{TC._split_seq}",
                                engine=ins.engine,
                                ins=[],
                                outs=[],
                            )
                            nop.sync_info = bass_rust.SyncInfo(
                                on_wait=[w], on_update=[]
                            )
                            out.append(nop)
                        ins.sync_info = bass_rust.SyncInfo(
                            on_wait=[waits[-1]], on_update=list(si.on_update)
                        )
                    out.append(ins)
                if changed:
                    blk.instructions = out
        return ret


def _r(ap):
    return ap.bitcast(F32R)


def build_program(split_waits=True):
    nc = bass.Bass("TRN2", target_bir_lowering=False, debug=False)
    dt = nc.dram_tensor
    xt_d = dt("xt", [C, S], F32R, kind="ExternalInput").ap()
    wqk_d = dt("wqk", [16, P, 8, P], F32R, kind="ExternalInput").ap()
    wv_d = dt("wv", [8, P, C], F32R, kind="ExternalInput").ap()
    wo_d = dt("wo", [P, 8, 8, P], F32, kind="ExternalInput").ap()
    gwp_d = dt("gwp", [P, 8, 65], F32R, kind="ExternalInput").ap()
    gb_d = dt("gb", [H, 1], F32, kind="ExternalInput").ap()
    cosf_d = dt("cosf", [P, S], BF16, kind="ExternalInput").ap()
    sinp_d = dt("sinp", [P, S], BF16, kind="ExternalInput").ap()
    maskt_d = dt("maskt", [P, P], BF16, kind="ExternalInput").ap()
    bonesp_d = dt("bonesp", [P, 65], BF16, kind="ExternalInput").ap()
    outt_d = dt("outt", [C, S], F32, kind="ExternalOutput").ap()
    if KDBG:
        dbgq_d = dt("dbgq", [P, 8, S], BF16, kind="ExternalOutput").ap()
        dbgk_d = dt("dbgk", [P, 8, S], BF16, kind="ExternalOutput").ap()
        dbgv_d = dt("dbgv", [P, 8, H * 65], BF16, kind="ExternalOutput").ap()
        dbgs_d = dt("dbgs", [H, S], F32, kind="ExternalOutput").ap()
        dbgsc_d = dt("dbgsc", [H, S], F32, kind="ExternalOutput").ap()
        dbga_d = dt("dbga", [P, 8, S], BF16, kind="ExternalOutput").ap()
    srt_scr = dt("srt_scr", [32, S], F32).ap()
    sums_scr = dt("sums_scr", [H, S], F32).ap()

    with TC(nc) as tc:
        tc.split_waits = split_waits
        with (
            tc.tile_pool(name="const", bufs=1) as constp,
            tc.tile_pool(name="resid", bufs=1) as resid,
            tc.tile_pool(name="stats", bufs=1) as stats,
        ):
            wvp = tc.alloc_tile_pool(name="wvp", bufs=1)
            cosf = constp.tile([P, S], BF16, tag="cosf")
            sinp = constp.tile([P, S], BF16, tag="sinp")
            maskt = constp.tile([P, P], BF16, tag="maskt")
            bonesp = constp.tile([P, 65], BF16, tag="bonesp")
            gwp_sb = constp.tile([P, 8, 65], F32R, tag="gwp")
            gb_sb = constp.tile([H, 1], F32, tag="gb")
            nc.sync.dma_start(cosf[:], cosf_d[:])
            nc.sync.dma_start(sinp[:], sinp_d[:])
            nc.sync.dma_start(maskt[:], maskt_d[:])
            nc.sync.dma_start(bonesp[:], bonesp_d[:])
            nc.sync.dma_start(gwp_sb[:], gwp_d[:])
            nc.sync.dma_start(gb_sb[:], gb_d[:])

            # v weights prefetched up front (fresh SBUF region, no waits)
            wv_sb = wvp.tile([P, 8, C], F32R, tag="wv")
            for c in range(8):
                nc.sync.dma_start(wv_sb[:, c, :], wv_d[c])

            qr = resid.tile([P, 8, S], BF16, tag="qr")
            kr = resid.tile([P, 8, S], BF16, tag="kr")
            # per k-tile: 16 heads x (64 v-cols + ones col)
            vaug = resid.tile([P, 8, H * 65], BF16, tag="vaug")
            aos = resid.tile([P, 8, S], BF16, tag="aos")

            gate_sb = stats.tile([H, S], F32, tag="gate")
            sums = stats.tile([H, S], F32, tag="sums")

            srt = stats.tile([32, S], F32, tag="srt")

            eps2q = stats.tile([2, 1], F32, tag="eps2q")
            eps2k = stats.tile([2, 1], F32, tag="eps2k")
            nc.vector.memset(eps2q[:], 1e-6)
            nc.vector.memset(eps2k[:], 6.4e-5)

            # ones columns of v_aug (col 64 of each head's 65-wide block)
            for kt in range(8):
                ones_ap = vaug[:, kt, :].rearrange("p (h e) -> p h e", h=H)[
                    :, :, 64:65
                ]
                nc.vector.memset(ones_ap, 1.0)

            xp = tc.alloc_tile_pool(name="xp", bufs=1)
            xT = xp.tile([P, 8, S], F32R, tag="xT")
            for c in range(8):
                nc.sync.dma_start(xT[:, c, :], xt_d[c * P : (c + 1) * P, :])

            # ---------------- phase 1a: q/k projections ----------------
            with (
                tc.tile_pool(name="wqks", bufs=2) as wqks,
                tc.tile_pool(name="work", bufs=2) as work,
                tc.tile_pool(name="stg2", bufs=2) as stg2p,
                tc.tile_pool(name="pq", bufs=2, space="PSUM") as pqp,
                tc.tile_pool(name="pbones", bufs=2, space="PSUM") as pbonesp,
            ):
                for f in range(16):
                    wt = wqks.tile([P, 8, P], F32R, tag="wt")
                    nc.sync.dma_start(wt[:], wqk_d[f])
                    pq = pqp.tile([P, S], F32, tag="pq")
                    for c in range(8):
                        for ch in range(2):
                            sl = slice(ch * 512, (ch + 1) * 512)
                            nc.tensor.matmul(
                                pq[:, sl],
                                _r(wt[:, c, :]),
                                _r(xT[:, c, sl]),
                                start=(c == 0),
                                stop=(c == 7),
                            )
                    dst = qr if f < 8 else kr
                    t = f % 8
                    # evacuate to bf16 on ACT, rotary in bf16 on DVE
                    pqs = work.tile([P, S], BF16, tag="pqs")
                    nc.scalar.activation(pqs[:], pq[:], AF.Copy)
                    tmp = work.tile([P, S], BF16, tag="tmp")
                    nc.vector.tensor_mul(dst[:, t, :], pqs[:], cosf[:])
                    for hl in range(2):
                        b0 = hl * 64
                        nc.vector.tensor_mul(
                            tmp[b0 : b0 + 32, :],
                            pqs[b0 + 32 : b0 + 64, :],
                            sinp[b0 + 32 : b0 + 64, :],
                        )
                        nc.vector.tensor_mul(
                            tmp[b0 + 32 : b0 + 64, :],
                            pqs[b0 : b0 + 32, :],
                            sinp[b0 : b0 + 32, :],
                        )
                    nc.vector.tensor_add(dst[:, t, :], dst[:, t, :], tmp[:])
                    # sum of squares over D per head: Square on ACT -> bones MM
                    sq = work.tile([P, S], BF16, tag="sq")
                    nc.scalar.activation(sq[:], dst[:, t, :], AF.Square)
                    ro = (0 if f < 8 else 16) + 2 * t
                    for ch in range(2):
                        sl = slice(ch * 512, (ch + 1) * 512)
                        pb = pbonesp.tile([65, 512], F32, tag="pb")
                        nc.tensor.matmul(pb[:], bonesp[:], sq[:, sl])
                        s2 = stg2p.tile([2, 512], F32, tag="s2")
                        if f < 8:
                            nc.scalar.activation(
                                s2[:], pb[0:2, :], AF.Sqrt, bias=eps2q[:, 0:1],
                                scale=1.0 / 64,
                            )
                        else:
                            nc.scalar.activation(
                                s2[:], pb[0:2, :], AF.Sqrt, bias=eps2k[:, 0:1],
                                scale=1.0,
                            )
                        nc.sync.dma_start(srt[ro : ro + 2, sl], s2[:])

            # ---------------- phase 1b: v (natural layout) + gate ----------
            with (
                tc.tile_pool(name="bcp", bufs=2) as bcp,
                tc.tile_pool(name="pvn", bufs=4, space="PSUM") as pvnp,
                tc.tile_pool(name="pgate", bufs=2, space="PSUM") as pgatep,
            ):
                # gate logits, one 512-chunk at a time (padded 65-col weights)
                for ch in range(2):
                    sl = slice(ch * 512, (ch + 1) * 512)
                    pgate = pgatep.tile([65, 512], F32, tag="pgate")
                    for c in range(8):
                        nc.tensor.matmul(
                            pgate[:],
                            gwp_sb[:, c, :],
                            _r(xT[:, c, sl]),
                            start=(c == 0),
                            stop=(c == 7),
                        )
                    nc.scalar.activation(
                        gate_sb[:, sl], pgate[0:H, :], AF.Sigmoid,
                        bias=gb_sb[:, 0:1],
                    )

                # v[tok, vfeat]: stationary = x tile, moving = Wv chunk
                for tt in range(8):
                    tsl = slice(tt * P, (tt + 1) * P)
                    pva = pvnp.tile([P, 512], F32, tag="pv")
                    pvb = pvnp.tile([P, 512], F32, tag="pv")
                    for c in range(8):
                        nc.tensor.matmul(
                            pva[:],
                            _r(xT[:, c, tsl]),
                            _r(wv_sb[:, c, 0:512]),
                            start=(c == 0),
                            stop=(c == 7),
                        )
                        nc.tensor.matmul(
                            pvb[:],
                            _r(xT[:, c, tsl]),
                            _r(wv_sb[:, c, 512:1024]),
                            start=(c == 0),
                            stop=(c == 7),
                        )
                    vre = vaug[:, tt, :].rearrange("p (h e) -> p h e", h=H)
                    nc.vector.tensor_copy(
                        vre[:, 0:8, 0:64], pva[:].rearrange("p (h e) -> p h e", h=8)
                    )
                    nc.vector.tensor_copy(
                        vre[:, 8:16, 0:64], pvb[:].rearrange("p (h e) -> p h e", h=8)
                    )

                # rms scales: reciprocal + broadcast + apply (overlaps v MMs)
                nc.vector.reciprocal(srt[:], srt[:])
                nc.sync.dma_start(srt_scr[:, :], srt[:])
                for side, dst in ((0, qr), (16, kr)):
                    for t in range(8):
                        bcf = bcp.tile([P, S], F32, tag="bcf")
                        for hl in range(2):
                            ro = side + 2 * t + hl
                            nc.sync.dma_start(
                                bcf[hl * 64 : (hl + 1) * 64, :],
                                srt_scr[ro : ro + 1, :].broadcast_to([64, S]),
                            )
                        bc = bcp.tile([P, S], BF16, tag="bc")
                        nc.vector.tensor_copy(bc[:], bcf[:])
                        nc.vector.tensor_mul(dst[:, t, :], dst[:, t, :], bc[:])
            xp.release()
            wvp.release()
            if KDBG:
                nc.sync.dma_start(dbgq_d[:], qr[:])
                nc.sync.dma_start(dbgk_d[:], kr[:])
                nc.sync.dma_start(dbgv_d[:], vaug[:])

            # ---------------- phases 2+3 ----------------
            if KPHASE < 2:
                nc.sync.dma_start(outt_d[:, :], qr[:].bitcast(F32))
                return nc
            # ---------------- phase 2: attention ----------------
            with (
                tc.tile_pool(name="expp", bufs=18) as expp,
                tc.tile_pool(name="bc2", bufs=2) as bc2p,
                tc.tile_pool(name="st2", bufs=3) as st2p,
                tc.tile_pool(name="ps", bufs=2, space="PSUM") as psp,
                tc.tile_pool(name="po", bufs=2, space="PSUM") as pop,
            ):
                for t in range(8):  # head pair: heads 2t (rows 0:64), 2t+1
                    poA = pop.tile([65, S], F32, tag="po")
                    poB = pop.tile([65, S], F32, tag="po")
                    pos = [poA, poB]
                    etss = [[], []]
                    for kt in range(8):
                        q0 = kt * P
                        nsp = S - q0
                        for hl in range(2):
                            r0 = hl * 64
                            ps = psp.tile([P, S], F32, tag="ps")
                            ofs = 0
                            while ofs < nsp:
                                n = min(512, nsp - ofs)
                                nc.tensor.matmul(
                                    ps[:, ofs : ofs + n],
                                    kr[r0 : r0 + 64, t, q0 : q0 + P],
                                    qr[r0 : r0 + 64, t, q0 + ofs : q0 + ofs + n],
                                )
                                ofs += n
                            et = expp.tile([P, S], BF16, tag="et")
                            nc.scalar.activation(
                                et[:, 0:nsp], ps[:, 0:nsp], AF.Exp
                            )
                            # causal mask on the diagonal tile
                            nc.vector.tensor_mul(
                                et[:, 0:P], et[:, 0:P], maskt[:]
                            )
                            etss[hl].append((et, nsp, q0))
                    for hl in range(2):
                        h = 2 * t + hl
                        po = pos[hl]
                        for kt in range(8):
                            et, nsp, q0 = etss[hl][kt]
                            ofs = 0
                            while ofs < nsp:
                                a = q0 + ofs
                                n = min(512 - (a % 512), nsp - ofs)
                                nc.tensor.matmul(
                                    po[:, a : a + n],
                                    vaug[:, kt, h * 65 : (h + 1) * 65],
                                    et[:, ofs : ofs + n],
                                    start=(kt == 0),
                                    stop=(kt == 4 * (a // 512) + 3),
                                )
                                ofs += n
                        # denominator row -> sums[h] via base-0 staging + DMA
                        s1 = st2p.tile([1, S], F32, tag="sd")
                        nc.vector.tensor_copy(s1[:], po[64:65, :])
                        nc.sync.dma_start(sums[h : h + 1, :], s1[:])
                        nc.vector.tensor_copy(
                            aos[hl * 64 : hl * 64 + 64, t, :], po[0:64, :]
                        )

                if KDBG:
                    nc.sync.dma_start(dbgs_d[:], sums[:])
                # scale = gate / denominator, applied per (channel, tok-half)
                nc.vector.reciprocal(sums[:], sums[:])
                nc.vector.tensor_mul(sums[:], sums[:], gate_sb[:])
                nc.sync.dma_start(sums_scr[:, :], sums[:])
                for ct in range(8):
                    bcf = bc2p.tile([P, S], F32, tag="bc2f")
                    for hl in range(2):
                        ro = 2 * ct + hl
                        nc.sync.dma_start(
                            bcf[hl * 64 : (hl + 1) * 64, :],
                            sums_scr[ro : ro + 1, :].broadcast_to([64, S]),
                        )
                    bc = bc2p.tile([P, S], BF16, tag="bc2")
                    nc.vector.tensor_copy(bc[:], bcf[:])
                    for ch in range(2):
                        sl = slice(ch * 512, (ch + 1) * 512)
                        nc.vector.tensor_mul(
                            aos[:, ct, sl], aos[:, ct, sl], bc[:, sl]
                        )

            if KDBG:
                nc.sync.dma_start(dbgsc_d[:], sums[:])
                for _ct in range(8):
                    nc.sync.dma_start(dbga_d[:, _ct, :], aos[:, _ct, :])
            if KPHASE < 3:
                return nc
            # ---------------- phase 3: output projection ----------------
            with (
                tc.tile_pool(name="wop", bufs=1) as wop,
                tc.tile_pool(name="wof", bufs=2) as wofp,
                tc.tile_pool(name="osb", bufs=3) as osbp,
                tc.tile_pool(name="pw", bufs=3, space="PSUM") as pwp,
            ):
                wos = wop.tile([P, 8, 8, P], BF16, tag="wo")
                for o in range(0, 8, 2):
                    wof = wofp.tile([P, 2, 8, P], F32, tag="wof")
                    nc.sync.dma_start(wof[:], wo_d[:, o : o + 2, :, :])
                    nc.vector.tensor_copy(wos[:, o : o + 2, :, :], wof[:])
                for ch in range(2):
                    sl = slice(ch * 512, (ch + 1) * 512)
                    for o in range(8):
                        pw = pwp.tile([P, 512], F32, tag="pw")
                        for c in range(8):
                            nc.tensor.matmul(
                                pw[:],
                                wos[:, o, c, :],
                                aos[:, c, sl],
                                start=(c == 0),
                                stop=(c == 7),
                            )
                        ot = osbp.tile([P, 512], F32, tag="ot")
                        nc.scalar.activation(ot[:], pw[:], AF.Copy)
                        nc.sync.dma_start(
                            outt_d[o * P : (o + 1) * P, sl], ot[:]
                        )
    return nc


def prepare_inputs(x, Wqkv, Wo, gate_w, gate_b, cos_cache, sin_cache, position_ids):
    """Host-side sharding + layout prep. Returns per-core input maps."""
    import ml_dtypes

    x = np.asarray(x, dtype=np.float32)
    WqkvT = np.asarray(Wqkv, dtype=np.float32).T  # [C, 3C]
    wqk_r = np.ascontiguousarray(
        WqkvT[:, 0:2048].reshape(8, P, 16, P).transpose(2, 1, 0, 3)
    )  # [f, p, c, d] for q,k only
    wv_r = np.ascontiguousarray(WqkvT[:, 2048:3072].reshape(8, P, C))
    WoT = np.asarray(Wo, dtype=np.float32).T  # [C, C]
    wo_r = np.ascontiguousarray(
        WoT.reshape(8, P, 8, P).transpose(1, 2, 0, 3)
    )
    gwT = np.asarray(gate_w, dtype=np.float32).T  # [C, H]
    gwp_r = np.zeros((P, 8, 65), dtype=np.float32)
    gwp_r[:, :, 0:H] = gwT.reshape(8, P, H).transpose(1, 0, 2)
    gb_r = np.asarray(gate_b, dtype=np.float32).reshape(H, 1)

    maskt = np.triu(np.ones((P, P), dtype=np.float32)).astype(ml_dtypes.bfloat16)
    bonesp = np.zeros((P, 65), dtype=np.float32)
    bonesp[0:64, 0] = 1.0
    bonesp[64:128, 1] = 1.0
    bonesp = bonesp.astype(ml_dtypes.bfloat16)
    cos_cache = np.asarray(cos_cache, dtype=np.float32)
    sin_cache = np.asarray(sin_cache, dtype=np.float32)
    position_ids = np.asarray(position_ids)

    in_maps = []
    for b in range(NCORES):
        xs = x[b * S : (b + 1) * S, :]
        pos = position_ids[b * S : (b + 1) * S]
        ct = cos_cache[pos].T  # [32, S]
        st = sin_cache[pos].T
        cosf = np.ascontiguousarray(np.tile(ct, (4, 1))).astype(ml_dtypes.bfloat16)
        sinp = np.ascontiguousarray(
            np.tile(np.concatenate([st, -st], axis=0), (2, 1))
        )
        sinp = np.ascontiguousarray(
            sinp.reshape(4, 32, S)[[1, 0, 3, 2]].reshape(P, S)
        ).astype(ml_dtypes.bfloat16)
        in_maps.append(
            {
                "xt": np.ascontiguousarray(xs.T),
                "wqk": wqk_r,
                "wv": wv_r,
                "wo": wo_r,
                "gwp": gwp_r,
                "gb": gb_r,
                "cosf": cosf,
                "sinp": sinp,
                "maskt": maskt,
                "bonesp": bonesp,
            }
        )
    return in_maps


_CACHED_NC = None


def kernel(
    x,
    Wqkv,
    Wo,
    gate_w,
    gate_b,
    cos_cache,
    sin_cache,
    cu_seqlens,
    position_ids,
    max_seqlen,
):
    global _CACHED_NC
    in_maps = prepare_inputs(
        x, Wqkv, Wo, gate_w, gate_b, cos_cache, sin_cache, position_ids
    )
    if _CACHED_NC is None:
        _CACHED_NC = build_program()
    res = bass_utils.run_bass_kernel_spmd(
        _CACHED_NC, in_maps, core_ids=list(range(NCORES))
    )
    out = np.empty((NCORES * S, C), dtype=np.float32)
    for b in range(NCORES):
        out[b * S : (b + 1) * S, :] = res.results[b]["outt"].T
    return out


# revision 18
# speedup vs baseline: 1.8222x; 1.0413x over previous
"""Causal varlen self-attention (qk-norm + rotary + head gating) on 8 trn2 cores.

Sharding: data-parallel by sequence - 8 packed equal-length sequences, one per
NeuronCore; weights replicated. No collectives.

Per-core dataflow (S=1024 tokens, C=1024 hidden, H=16 heads, D=64):
  phase 1a: q/k computed TRANSPOSED ([feat, tok]) so scores need no transposes.
            PSUM evacuated to bf16 by ACT; rotary + rms-norm run in bf16 on
            DVE (2x mode); sum-of-squares on ACT (Square) feeding a 65-col
            padded "bones" stationary so the PE never leaves 128x128 mode.
  phase 1b: v computed in NATURAL [tok, D] layout directly (x-tile stationary,
            Wv moving) - no PE transposes. Ones column appended per head so
            the softmax denominator falls out of the PV matmul. gate =
            sigmoid(gwp @ x^T + b) with gwp padded to 65 cols. rms scales are
            applied while v runs on the PE.
  phase 2:  per head PAIR: row-tiled scores (heads 2t/2t+1 run concurrently
            on PE row-groups 0/64), exp on ACT into bf16 et tiles
            ([128,1024] PSUM reads = fat ACT instrs), causal mask on the
            diagonal tile, then PV bursts (bf16, 65-col stationary; row 64 =
            denominators). Normalization + gating applied as one broadcast
            multiply per token-half on the accumulated attention output.
  phase 3:  out^T = Wo^T-tiles-stationary x ao-moving, split by token halves
            so Wo overlaps the gating tail; host transposes back.

Projections/bones run fp32r; q/k/scores/PV/Wo run bf16 (fp32 PSUM accum).
Compute-engine APs must start at partition 0/32/64/96; per-head stat rows are
routed through base-0 staging tiles + SBUF-to-SBUF DMA (which is unrestricted).
"""

import sys

sys.path.insert(0, "/opt/trn_rl_repo")

import numpy as np
import bass_rust
import concourse.bass as bass
import concourse.tile as tile
from concourse import mybir
from concourse import bass_utils

import os
KPHASE = int(os.environ.get("KPHASE", "3"))
KDBG = int(os.environ.get("KDBG", "0"))

P = 128
S = 1024  # tokens per sequence (= per core)
C = 1024  # hidden
H = 16
D = 64
NCORES = 8
F32 = mybir.dt.float32
F32R = mybir.dt.float32r
BF16 = mybir.dt.bfloat16
AF = mybir.ActivationFunctionType


class TC(tile.TileContext):
    """TileContext that rewrites every instruction to carry at most ONE sem wait.

    This container's walrus rejects instructions with more than one sync wait
    command (matmul LDW structs, CTRL drains, ...). Tile's wait-assignment
    pass attaches one wait per producer proc, so fan-in instructions get
    several. After scheduling, hoist all but the last wait of each
    instruction onto same-engine NOPs inserted immediately before it -
    identical synchronization semantics, one wait per encoded instruction.
    """

    _split_seq = 0
    split_waits = True

    def schedule_and_allocate(self, *args, **kwargs):
        ret = super().schedule_and_allocate(*args, **kwargs)
        if not self.split_waits:
            return ret
        nc = self.nc
        for fn in nc.m.functions:
            for blk in fn.blocks:
                insts = blk.instructions
                out = []
                changed = False
                for ins in insts:
                    si = getattr(ins, "sync_info", None)
                    waits = list(si.on_wait) if si is not None else []
                    if len(waits) > 1:
                        changed = True
                        for w in waits[:-1]:
                            TC._split_seq += 1
                            nop = bass_rust.InstNoOp(
                                name=f"I-splitw-{TC._split_seq}",
                                engine=ins.engine,
                                ins=[],
                                outs=[],
                            )
                            nop.sync_info = bass_rust.SyncInfo(
                                on_wait=[w], on_update=[]
                            )
                            out.append(nop)
                        ins.sync_info = bass_rust.SyncInfo(
                            on_wait=[waits[-1]], on_update=list(si.on_update)
                        )
                    out.append(ins)
                if changed:
                    blk.instructions = out
        return ret


def _r(ap):
    return ap.bitcast(F32R)


def build_program(split_waits=True):
    nc = bass.Bass("TRN2", target_bir_lowering=False, debug=False)
    dt = nc.dram_tensor
    xt_d = dt("xt", [C, S], F32R, kind="ExternalInput").ap()
    wqk_d = dt("wqk", [16, P, 8, P], F32R, kind="ExternalInput").ap()
    wv_d = dt("wv", [8, P, C], F32R, kind="ExternalInput").ap()
    wo_d = dt("wo", [P, 8, 8, P], F32, kind="ExternalInput").ap()
    gwp_d = dt("gwp", [P, 8, 65], F32R, kind="ExternalInput").ap()
    gb_d = dt("gb", [H, 1], F32, kind="ExternalInput").ap()
    cosf_d = dt("cosf", [P, S], BF16, kind="ExternalInput").ap()
    sinp_d = dt("sinp", [P, S], BF16, kind="ExternalInput").ap()
    maskt_d = dt("maskt", [P, P], BF16, kind="ExternalInput").ap()
    bonesp_d = dt("bonesp", [P, 65], BF16, kind="ExternalInput").ap()
    outt_d = dt("outt", [C, S], F32, kind="ExternalOutput").ap()
    if KDBG:
        dbgq_d = dt("dbgq", [P, 8, S], BF16, kind="ExternalOutput").ap()
        dbgk_d = dt("dbgk", [P, 8, S], BF16, kind="ExternalOutput").ap()
        dbgv_d = dt("dbgv", [P, 8, H * 65], BF16, kind="ExternalOutput").ap()
        dbgs_d = dt("dbgs", [H, S], F32, kind="ExternalOutput").ap()
        dbgsc_d = dt("dbgsc", [H, S], F32, kind="ExternalOutput").ap()
        dbga_d = dt("dbga", [P, 8, S], BF16, kind="ExternalOutput").ap()
    srt_scr = dt("srt_scr", [32, S], F32).ap()
    sums_scr = dt("sums_scr", [H, S], F32).ap()

    with TC(nc) as tc:
        tc.split_waits = split_waits
        with (
            tc.tile_pool(name="const", bufs=1) as constp,
            tc.tile_pool(name="resid", bufs=1) as resid,
            tc.tile_pool(name="stats", bufs=1) as stats,
        ):
            wvp = tc.alloc_tile_pool(name="wvp", bufs=1)
            cosf = constp.tile([P, S], BF16, tag="cosf")
            sinp = constp.tile([P, S], BF16, tag="sinp")
            maskt = constp.tile([P, P], BF16, tag="maskt")
            bonesp = constp.tile([P, 65], BF16, tag="bonesp")
            gwp_sb = constp.tile([P, 8, 65], F32R, tag="gwp")
            gb_sb = constp.tile([H, 1], F32, tag="gb")
            nc.sync.dma_start(cosf[:], cosf_d[:])
            nc.sync.dma_start(sinp[:], sinp_d[:])
            nc.sync.dma_start(maskt[:], maskt_d[:])
            nc.sync.dma_start(bonesp[:], bonesp_d[:])
            nc.sync.dma_start(gwp_sb[:], gwp_d[:])
            nc.sync.dma_start(gb_sb[:], gb_d[:])

            # v weights prefetched up front (fresh SBUF region, no waits)
            wv_sb = wvp.tile([P, 8, C], F32R, tag="wv")
            for c in range(8):
                nc.sync.dma_start(wv_sb[:, c, :], wv_d[c])

            qr = resid.tile([P, 8, S], BF16, tag="qr")
            kr = resid.tile([P, 8, S], BF16, tag="kr")
            # per k-tile: 16 heads x (64 v-cols + ones col)
            vaug = resid.tile([P, 8, H * 65], BF16, tag="vaug")
            aos = resid.tile([P, 8, S], BF16, tag="aos")

            gate_sb = stats.tile([H, S], F32, tag="gate")
            sums = stats.tile([H, S], F32, tag="sums")

            srt = stats.tile([32, S], F32, tag="srt")

            eps2q = stats.tile([2, 1], F32, tag="eps2q")
            eps2k = stats.tile([2, 1], F32, tag="eps2k")
            nc.vector.memset(eps2q[:], 1e-6)
            nc.vector.memset(eps2k[:], 6.4e-5)

            # ones columns of v_aug (col 64 of each head's 65-wide block)
            for kt in range(8):
                ones_ap = vaug[:, kt, :].rearrange("p (h e) -> p h e", h=H)[
                    :, :, 64:65
                ]
                nc.vector.memset(ones_ap, 1.0)

            xp = tc.alloc_tile_pool(name="xp", bufs=1)
            xT = xp.tile([P, 8, S], F32R, tag="xT")
            for c in range(8):
                nc.sync.dma_start(xT[:, c, :], xt_d[c * P : (c + 1) * P, :])

            # ---------------- phase 1a: q/k projections ----------------
            with (
                tc.tile_pool(name="wqks", bufs=2) as wqks,
                tc.tile_pool(name="work", bufs=3) as work,
                tc.tile_pool(name="stg2", bufs=4) as stg2p,
                tc.tile_pool(name="pq", bufs=3, space="PSUM") as pqp,
                tc.tile_pool(name="pbones", bufs=2, space="PSUM") as pbonesp,
            ):
                for f in range(16):
                    wt = wqks.tile([P, 8, P], F32R, tag="wt")
                    nc.sync.dma_start(wt[:], wqk_d[f])
                    pq = pqp.tile([P, S], F32, tag="pq")
                    for c in range(8):
                        for ch in range(2):
                            sl = slice(ch * 512, (ch + 1) * 512)
                            nc.tensor.matmul(
                                pq[:, sl],
                                _r(wt[:, c, :]),
                                _r(xT[:, c, sl]),
                                start=(c == 0),
                                stop=(c == 7),
                            )
                    dst = qr if f < 8 else kr
                    t = f % 8
                    # evacuate to bf16 on ACT, rotary in bf16 on DVE
                    pqs = work.tile([P, S], BF16, tag="pqs")
                    nc.scalar.activation(pqs[:], pq[:], AF.Copy)
                    tmp = work.tile([P, S], BF16, tag="tmp")
                    nc.vector.tensor_mul(dst[:, t, :], pqs[:], cosf[:])
                    for hl in range(2):
                        b0 = hl * 64
                        nc.vector.tensor_mul(
                            tmp[b0 : b0 + 32, :],
                            pqs[b0 + 32 : b0 + 64, :],
                            sinp[b0 + 32 : b0 + 64, :],
                        )
                        nc.vector.tensor_mul(
                            tmp[b0 + 32 : b0 + 64, :],
                            pqs[b0 : b0 + 32, :],
                            sinp[b0 : b0 + 32, :],
                        )
                    nc.vector.tensor_add(dst[:, t, :], dst[:, t, :], tmp[:])
                    # sum of squares over D per head: Square on ACT -> bones MM
                    sq = work.tile([P, S], BF16, tag="sq")
                    nc.scalar.activation(sq[:], dst[:, t, :], AF.Square)
                    ro = (0 if f < 8 else 16) + 2 * t
                    for ch in range(2):
                        sl = slice(ch * 512, (ch + 1) * 512)
                        pb = pbonesp.tile([65, 512], F32, tag="pb")
                        nc.tensor.matmul(pb[:], bonesp[:], sq[:, sl])
                        s2 = stg2p.tile([2, 512], F32, tag="s2")
                        if f < 8:
                            nc.scalar.activation(
                                s2[:], pb[0:2, :], AF.Ln,
                                bias=eps2q[:, 0:1], scale=1.0 / 64,
                            )
                        else:
                            nc.scalar.activation(
                                s2[:], pb[0:2, :], AF.Ln,
                                bias=eps2k[:, 0:1], scale=1.0,
                            )
                        nc.sync.dma_start(srt[ro : ro + 2, sl], s2[:])

            # ---------------- phase 1b: v (natural layout) + gate ----------
            with (
                tc.tile_pool(name="bcp", bufs=2) as bcp,
                tc.tile_pool(name="pvn", bufs=4, space="PSUM") as pvnp,
                tc.tile_pool(name="pgate", bufs=2, space="PSUM") as pgatep,
            ):
                # gate logits, one 512-chunk at a time (padded 65-col weights)
                for ch in range(2):
                    sl = slice(ch * 512, (ch + 1) * 512)
                    pgate = pgatep.tile([65, 512], F32, tag="pgate")
                    for c in range(8):
                        nc.tensor.matmul(
                            pgate[:],
                            gwp_sb[:, c, :],
                            _r(xT[:, c, sl]),
                            start=(c == 0),
                            stop=(c == 7),
                        )
                    nc.scalar.activation(
                        gate_sb[:, sl], pgate[0:H, :], AF.Sigmoid,
                        bias=gb_sb[:, 0:1],
                    )

                # v[tok, vfeat]: stationary = x tile, moving = Wv chunk
                for tt in range(8):
                    tsl = slice(tt * P, (tt + 1) * P)
                    pva = pvnp.tile([P, 512], F32, tag="pv")
                    pvb = pvnp.tile([P, 512], F32, tag="pv")
                    for c in range(8):
                        nc.tensor.matmul(
                            pva[:],
                            _r(xT[:, c, tsl]),
                            _r(wv_sb[:, c, 0:512]),
                            start=(c == 0),
                            stop=(c == 7),
                        )
                        nc.tensor.matmul(
                            pvb[:],
                            _r(xT[:, c, tsl]),
                            _r(wv_sb[:, c, 512:1024]),
                            start=(c == 0),
                            stop=(c == 7),
                        )
                    vre = vaug[:, tt, :].rearrange("p (h e) -> p h e", h=H)
                    nc.vector.tensor_copy(
                        vre[:, 0:8, 0:64], pva[:].rearrange("p (h e) -> p h e", h=8)
                    )
                    nc.vector.tensor_copy(
                        vre[:, 8:16, 0:64], pvb[:].rearrange("p (h e) -> p h e", h=8)
                    )

                # 1/sqrt via exp(-0.5 ln) on ACT
                srt2 = stats.tile([32, S], F32, tag="srt2")
                nc.scalar.activation(srt2[:], srt[:], AF.Exp, scale=-0.5)
                nc.sync.dma_start(srt_scr[:, :], srt2[:])
                for t in range(8):
                    for side, dst in ((0, qr), (16, kr)):
                        bcf = bcp.tile([P, S], F32, tag="bcf")
                        for hl in range(2):
                            ro = side + 2 * t + hl
                            nc.sync.dma_start(
                                bcf[hl * 64 : (hl + 1) * 64, :],
                                srt_scr[ro : ro + 1, :].broadcast_to([64, S]),
                            )
                        bc = bcp.tile([P, S], BF16, tag="bc")
                        nc.vector.tensor_copy(bc[:], bcf[:])
                        nc.vector.tensor_mul(dst[:, t, :], dst[:, t, :], bc[:])
            xp.release()
            wvp.release()
            if KDBG:
                nc.sync.dma_start(dbgq_d[:], qr[:])
                nc.sync.dma_start(dbgk_d[:], kr[:])
                nc.sync.dma_start(dbgv_d[:], vaug[:])

            # ---------------- phases 2+3 ----------------
            if KPHASE < 2:
                nc.sync.dma_start(outt_d[:, :], qr[:].bitcast(F32))
                return nc
            # ---------------- phase 2: attention ----------------
            with (
                tc.tile_pool(name="expp", bufs=18) as expp,
                tc.tile_pool(name="bc2", bufs=3) as bc2p,
                tc.tile_pool(name="st2", bufs=3) as st2p,
                tc.tile_pool(name="ps", bufs=2, space="PSUM") as psp,
                tc.tile_pool(name="po", bufs=2, space="PSUM") as pop,
            ):
                for t in range(8):  # head pair: heads 2t (rows 0:64), 2t+1
                    poA = pop.tile([65, S], F32, tag="po")
                    poB = pop.tile([65, S], F32, tag="po")
                    pos = [poA, poB]
                    etss = [[], []]
                    for kt in range(8):
                        q0 = kt * P
                        nsp = S - q0
                        for hl in range(2):
                            r0 = hl * 64
                            ps = psp.tile([P, S], F32, tag="ps")
                            ofs = 0
                            while ofs < nsp:
                                n = min(512, nsp - ofs)
                                nc.tensor.matmul(
                                    ps[:, ofs : ofs + n],
                                    kr[r0 : r0 + 64, t, q0 : q0 + P],
                                    qr[r0 : r0 + 64, t, q0 + ofs : q0 + ofs + n],
                                )
                                ofs += n
                            et = expp.tile([P, S], BF16, tag="et")
                            nc.scalar.activation(
                                et[:, 0:nsp], ps[:, 0:nsp], AF.Exp
                            )
                            # causal mask on the diagonal tile
                            nc.vector.tensor_mul(
                                et[:, 0:P], et[:, 0:P], maskt[:]
                            )
                            etss[hl].append((et, nsp, q0))
                    for hl in range(2):
                        h = 2 * t + hl
                        po = pos[hl]
                        for kt in range(8):
                            et, nsp, q0 = etss[hl][kt]
                            ofs = 0
                            while ofs < nsp:
                                a = q0 + ofs
                                n = min(512 - (a % 512), nsp - ofs)
                                nc.tensor.matmul(
                                    po[:, a : a + n],
                                    vaug[:, kt, h * 65 : (h + 1) * 65],
                                    et[:, ofs : ofs + n],
                                    start=(kt == 0),
                                    stop=(kt == 4 * (a // 512) + 3),
                                )
                                ofs += n
                        # denominator row -> sums[h] via base-0 staging + DMA
                        s1 = st2p.tile([1, S], F32, tag="sd")
                        nc.vector.tensor_copy(s1[:], po[64:65, :])
                        nc.sync.dma_start(sums[h : h + 1, :], s1[:])
                        nc.vector.tensor_copy(
                            aos[hl * 64 : hl * 64 + 64, t, :], po[0:64, :]
                        )

                if KDBG:
                    nc.sync.dma_start(dbgs_d[:], sums[:])
                # scale = gate / denominator: 1/d = exp(-ln d) on ACT
                suml = stats.tile([H, S], F32, tag="suml")
                nc.scalar.activation(suml[:], sums[:], AF.Ln)
                for ch in range(2):
                    sl = slice(ch * 512, (ch + 1) * 512)
                    nc.scalar.activation(
                        sums[:, sl], suml[:, sl], AF.Exp, scale=-1.0
                    )
                    nc.vector.tensor_mul(
                        sums[:, sl], sums[:, sl], gate_sb[:, sl]
                    )
                    nc.sync.dma_start(sums_scr[:, sl], sums[:, sl])
                    for ct in range(8):
                        bcf = bc2p.tile([P, 512], F32, tag="bc2f")
                        for hl in range(2):
                            ro = 2 * ct + hl
                            nc.sync.dma_start(
                                bcf[hl * 64 : (hl + 1) * 64, :],
                                sums_scr[ro : ro + 1, sl].broadcast_to([64, 512]),
                            )
                        bc = bc2p.tile([P, 512], BF16, tag="bc2")
                        nc.vector.tensor_copy(bc[:], bcf[:])
                        nc.vector.tensor_mul(
                            aos[:, ct, sl], aos[:, ct, sl], bc[:]
                        )

            if KDBG:
                nc.sync.dma_start(dbgsc_d[:], sums[:])
                for _ct in range(8):
                    nc.sync.dma_start(dbga_d[:, _ct, :], aos[:, _ct, :])
            if KPHASE < 3:
                return nc
            # ---------------- phase 3: output projection ----------------
            with (
                tc.tile_pool(name="wop", bufs=1) as wop,
                tc.tile_pool(name="wof", bufs=2) as wofp,
                tc.tile_pool(name="osb", bufs=3) as osbp,
                tc.tile_pool(name="pw", bufs=4, space="PSUM") as pwp,
            ):
                wos = wop.tile([P, 8, 8, P], BF16, tag="wo")
                for o in range(0, 8, 2):
                    wof = wofp.tile([P, 2, 8, P], F32, tag="wof")
                    nc.sync.dma_start(wof[:], wo_d[:, o : o + 2, :, :])
                    nc.vector.tensor_copy(wos[:, o : o + 2, :, :], wof[:])
                for ch in range(2):
                    sl = slice(ch * 512, (ch + 1) * 512)
                    for o in range(8):
                        pw = pwp.tile([P, 512], F32, tag="pw")
                        for c in range(8):
                            nc.tensor.matmul(
                                pw[:],
                                wos[:, o, c, :],
                                aos[:, c, sl],
                                start=(c == 0),
                                stop=(c == 7),
                            )
                        ot = osbp.tile([P, 512], F32, tag="ot")
                        nc.scalar.activation(ot[:], pw[:], AF.Copy)
                        nc.sync.dma_start(
                            outt_d[o * P : (o + 1) * P, sl], ot[:]
                        )
    return nc


def prepare_inputs(x, Wqkv, Wo, gate_w, gate_b, cos_cache, sin_cache, position_ids):
    """Host-side sharding + layout prep. Returns per-core input maps."""
    import ml_dtypes

    x = np.asarray(x, dtype=np.float32)
    WqkvT = np.asarray(Wqkv, dtype=np.float32).T  # [C, 3C]
    wqk_r = np.ascontiguousarray(
        WqkvT[:, 0:2048].reshape(8, P, 16, P).transpose(2, 1, 0, 3)
    )  # [f, p, c, d] for q,k only
    wv_r = np.ascontiguousarray(WqkvT[:, 2048:3072].reshape(8, P, C))
    WoT = np.asarray(Wo, dtype=np.float32).T  # [C, C]
    wo_r = np.ascontiguousarray(
        WoT.reshape(8, P, 8, P).transpose(1, 2, 0, 3)
    )
    gwT = np.asarray(gate_w, dtype=np.float32).T  # [C, H]
    gwp_r = np.zeros((P, 8, 65), dtype=np.float32)
    gwp_r[:, :, 0:H] = gwT.reshape(8, P, H).transpose(1, 0, 2)
    gb_r = np.asarray(gate_b, dtype=np.float32).reshape(H, 1)

    maskt = np.triu(np.ones((P, P), dtype=np.float32)).astype(ml_dtypes.bfloat16)
    bonesp = np.zeros((P, 65), dtype=np.float32)
    bonesp[0:64, 0] = 1.0
    bonesp[64:128, 1] = 1.0
    bonesp = bonesp.astype(ml_dtypes.bfloat16)
    cos_cache = np.asarray(cos_cache, dtype=np.float32)
    sin_cache = np.asarray(sin_cache, dtype=np.float32)
    position_ids = np.asarray(position_ids)

    in_maps = []
    for b in range(NCORES):
        xs = x[b * S : (b + 1) * S, :]
        pos = position_ids[b * S : (b + 1) * S]
        ct = cos_cache[pos].T  # [32, S]
        st = sin_cache[pos].T
        cosf = np.ascontiguousarray(np.tile(ct, (4, 1))).astype(ml_dtypes.bfloat16)
        sinp = np.ascontiguousarray(
            np.tile(np.concatenate([st, -st], axis=0), (2, 1))
        )
        sinp = np.ascontiguousarray(
            sinp.reshape(4, 32, S)[[1, 0, 3, 2]].reshape(P, S)
        ).astype(ml_dtypes.bfloat16)
        in_maps.append(
            {
                "xt": np.ascontiguousarray(xs.T),
                "wqk": wqk_r,
                "wv": wv_r,
                "wo": wo_r,
                "gwp": gwp_r,
                "gb": gb_r,
                "cosf": cosf,
                "sinp": sinp,
                "maskt": maskt,
                "bonesp": bonesp,
            }
        )
    return in_maps


_CACHED_NC = None


def kernel(
    x,
    Wqkv,
    Wo,
    gate_w,
    gate_b,
    cos_cache,
    sin_cache,
    cu_seqlens,
    position_ids,
    max_seqlen,
):
    global _CACHED_NC
    in_maps = prepare_inputs(
        x, Wqkv, Wo, gate_w, gate_b, cos_cache, sin_cache, position_ids
    )
    if _CACHED_NC is None:
        _CACHED_NC = build_program()
    res = bass_utils.run_bass_kernel_spmd(
        _CACHED_NC, in_maps, core_ids=list(range(NCORES))
    )
    out = np.empty((NCORES * S, C), dtype=np.float32)
    for b in range(NCORES):
        out[b * S : (b + 1) * S, :] = res.results[b]["outt"].T
    return out
